# revision 9
# baseline (speedup 1.0000x reference)
"""Trainium2 Bass kernel for nn_BiasedMHABlock (biased MHA + FFN transformer block).

Sharding: batch B=8 -> one batch per NeuronCore (SPMD, no collectives).

Per-core math (batch b), fully fused on-device:
  scores^T[w,u] per head = (K_h Q_h^T)/8 + CB[w,u] + relband_h[w,u], where
  softmax-invariant constants are dropped and
  CB = simscale*Xn Xn^T - gate*OneHot(spk) OneHot(spk)^T is built once via PE
  and added per-head with identity-matmul PSUM accumulation.
  Softmax runs over the partition axis without max-subtraction (scores are O(1));
  the denominator comes free from an appended ones-column of V in the attn@V
  matmul and is divided out post-hoc.
  Then X1 = LN(X+bo + Attn@Wo), FFN with transposed hidden, X2 = LN(X1+ffn).

All matmuls run as float32r (full-rate fp32); ffn2 runs bf16.
"""
import sys
import math

for _p in ("/opt/trn_rl_repo",):
    if _p not in sys.path:
        sys.path.insert(0, _p)

import numpy as np
import ml_dtypes

import concourse.bass as bass
import concourse.tile as tile
from concourse import bacc, mybir
from concourse.bass_utils import run_bass_kernel_spmd

F32 = mybir.dt.float32
F32R = mybir.dt.float32r
BF16 = mybir.dt.bfloat16
AF = mybir.ActivationFunctionType
ALU = mybir.AluOpType

B, U, D, H, DH, DFF = 8, 1024, 512, 8, 64, 4096
REL_MAX = 128
P = 128
NCORES = 8
LN_EPS = 1e-5
UBLK = 256  # ffn u-block

_prog_cache = {}


def _build_program(fast_gates: bool, apply_mask: bool, ncat: int):
    nc = bacc.Bacc("TRN2", target_bir_lowering=False, debug=False)

    def din(name, shape, dt=F32R):
        return nc.dram_tensor(name, list(shape), dt, kind="ExternalInput").ap()

    xt = din("xt", [4, P, U])
    xpbo = din("xpbo", [8, P, D])
    rns_a = din("rns_a", [1, U], F32)
    rns_b = din("rns_b", [1, U], F32)
    pta = din("pta", [H, ncat, U])
    ptb = din("ptb", [ncat, U])
    wq = din("wq", [4, P, D])
    wk = din("wk", [4, P, D])
    wv = din("wv", [5, P, D])
    wo = din("wo", [4, P, D])
    w1 = din("w1", [4, P, DFF])
    w2 = din("w2", [33, P, D], BF16)
    bf1p = din("bf1p", [P, 32], F32)
    qkb = din("qkb", [P, 8], F32)
    rbd = din("rbd", [P, H, 3, P])
    lnw = din("lnw", [4, D], F32)
    expd = din("expd", [16, 8 * P])
    identd = din("identd", [P, P])
    ones_pe = din("ones_pe", [1, P])
    ones_pb = din("ones_pb", [1, P], BF16)
    ones_v = din("ones_v", [P, 64])
    validd = din("validd", [P, 8], F32)
    if not fast_gates:
        sidents = din("sidents", [H, P, P])
        gidents = din("gidents", [H, P, P])

    out = nc.dram_tensor("out", [8, P, D], F32, kind="ExternalOutput").ap()
    dscratch = nc.dram_tensor("dscratch", [1, 16 * D], F32, kind="Internal").ap()

    open_cms = {}

    with tile.TileContext(nc) as tc, nc.allow_low_precision(reason="fp32r kernel"):
        def pool(name, bufs, space="SBUF", side="left"):
            cm = tc.tile_pool(name=name, bufs=bufs, space=space, side=side)
            p = cm.__enter__()
            open_cms[name] = cm
            return p

        def close(*names):
            for n in names:
                open_cms.pop(n).__exit__(None, None, None)

        try:
            # ---------------- constants (left, whole-kernel) ----------------
            consts = pool("consts", 1)
            ident = consts.tile([P, P], F32R)
            nc.sync.dma_start(ident, identd)
            qkb_t = consts.tile([P, 8], F32)
            nc.sync.dma_start(qkb_t, qkb)
            bf1_t = consts.tile([P, 32], F32)
            nc.sync.dma_start(bf1_t, bf1p)
            valid_t = consts.tile([P, 8], F32)
            nc.sync.dma_start(valid_t, validd)
            epst = consts.tile([P, 1], F32)
            nc.vector.memset(epst, LN_EPS)
            ones_u = consts.tile([1, P], F32R)
            nc.sync.dma_start(ones_u, ones_pe)
            ones_bf = consts.tile([1, P], BF16)
            nc.sync.dma_start(ones_bf, ones_pb)

            # ------------- long-lived attention inputs (left) ----------------
            attn_in = pool("attn_in", 1)
            qt_t = attn_in.tile([P, 4, U], F32R, tag="qt", name="qt")
            kt_t = attn_in.tile([P, 4, U], F32R, tag="kt", name="kt")
            vt_t = attn_in.tile([P, 8, 520], F32R, tag="vt", name="vt")
            rb_t = attn_in.tile([P, H, 3, P], F32R, tag="rbt", name="rbt")
            nc.sync.dma_start(rb_t, rbd)
            if fast_gates:
                cb_mats = [attn_in.tile([P, 8, U], F32R, tag="cbt", name="cbt")]
            else:
                cb_mats = [
                    attn_in.tile([P, 8, U], F32R, tag="simt", name="simt"),
                    attn_in.tile([P, 8, U], F32R, tag="eqt", name="eqt"),
                ]
                sid_t = attn_in.tile([H, P, P], F32R, tag="sid", name="sid")
                nc.sync.dma_start(sid_t, sidents)
                gid_t = attn_in.tile([H, P, P], F32R, tag="gid", name="gid")
                nc.sync.dma_start(gid_t, gidents)

            # ======================= PHASE 1: prep ==========================
            pre = pool("pre", 1, side="right")
            prew = pool("prew", 2, side="right")
            ps1 = pool("ps1", 2, space="PSUM")

            xt_t = pre.tile([P, 4, U], F32R, tag="xt", name="xtt")
            for c in range(4):
                nc.sync.dma_start(xt_t[:, c, :], xt[c])
            rnsa_b = pre.tile([P, U], F32, tag="rnsa", name="rnsa")
            nc.gpsimd.dma_start(
                rnsa_b, bass.AP(tensor=rns_a.tensor, offset=0, ap=[[0, P], [1, U]])
            )
            rnsb_b = pre.tile([P, U], F32, tag="rnsb", name="rnsb")
            nc.gpsimd.dma_start(
                rnsb_b, bass.AP(tensor=rns_b.tensor, offset=0, ap=[[0, P], [1, U]])
            )

            wq_t = prew.tile([P, 5, D], F32R, tag="wx", name="wqt")
            wk_t = prew.tile([P, 5, D], F32R, tag="wx", name="wkt")
            for c in range(4):
                nc.sync.dma_start(wq_t[:, c, :], wq[c])
                nc.sync.dma_start(wk_t[:, c, :], wk[c])

            # Q^T, K^T: psum[e_tile, u_half] = sum_c Wx[c]-slice^T @ XT
            for (wt, dst, boff) in ((wq_t, qt_t, 0), (wk_t, kt_t, 4)):
                for t in range(4):
                    for j in range(2):
                        ps = ps1.tile([P, D], F32, tag="psqk", name="psqk")
                        for c in range(4):
                            nc.tensor.matmul(
                                ps,
                                wt[:, c, t * P:(t + 1) * P],
                                xt_t[:, c, j * D:(j + 1) * D],
                                start=(c == 0), stop=(c == 3),
                            )
                        nc.scalar.activation(
                            dst[:, t, j * D:(j + 1) * D], ps, AF.Identity,
                            bias=qkb_t[:, boff + t:boff + t + 1],
                        )

            # V (interleaved (dh h) layout + ones cols)
            wv_t = prew.tile([P, 5, D], F32R, tag="wx", name="wvt")
            for c in range(5):
                nc.sync.dma_start(wv_t[:, c, :], wv[c])
            for t in range(8):
                ps = ps1.tile([P, D], F32, tag="psv", name="psv")
                for c in range(4):
                    nc.tensor.matmul(
                        ps, xt_t[:, c, t * P:(t + 1) * P], wv_t[:, c, :],
                        start=(c == 0), stop=False,
                    )
                nc.tensor.matmul(
                    ps, ones_u[0:1, :], wv_t[0:1, 4, :],
                    start=False, stop=True,
                )
                nc.vector.tensor_copy(
                    vt_t[:, t, 0:D].rearrange("p (dh h) -> p dh h", h=H),
                    ps.rearrange("p (h dh) -> p dh h", h=H),
                )
            nc.sync.dma_start(
                vt_t[:, :, D:520], ones_v.rearrange("p (t f) -> p t f", t=8)
            )
            if apply_mask:
                for t in range(8):
                    nc.vector.tensor_scalar_mul(
                        vt_t[:, t, :], vt_t[:, t, :], valid_t[:, t:t + 1],
                    )

            # Xn^T (scaled / unscaled sides) and CB (or SIM + EQ)
            xna_t = pre.tile([P, 4, U], F32R, tag="xna", name="xna")
            xnb_t = pre.tile([P, 4, U], F32R, tag="xnb", name="xnb")
            for c in range(4):
                if fast_gates:
                    nc.vector.tensor_tensor(
                        xna_t[:, c, :], xt_t[:, c, :], rnsa_b, ALU.mult
                    )
                nc.vector.tensor_tensor(
                    xnb_t[:, c, :], xt_t[:, c, :], rnsb_b, ALU.mult
                )

            npta = 1 if fast_gates else H
            pta_t = pre.tile([ncat, npta, U], F32R, tag="pta", name="ptat")
            ptb_t = pre.tile([ncat, U], F32R, tag="ptb", name="ptbt")
            nc.sync.dma_start(ptb_t, ptb)
            for h in range(npta):
                nc.sync.dma_start(pta_t[:, h, :], pta[h])

            if fast_gates:
                cbt = cb_mats[0]
                for i in range(8):
                    for j in range(2):
                        ps = ps1.tile([P, D], F32, tag="pscb", name="pscb")
                        for c in range(4):
                            nc.tensor.matmul(
                                ps,
                                xna_t[:, c, i * P:(i + 1) * P],
                                xnb_t[:, c, j * D:(j + 1) * D],
                                start=(c == 0), stop=False,
                            )
                        nc.tensor.matmul(
                            ps,
                            pta_t[:, 0, i * P:(i + 1) * P],
                            ptb_t[:, j * D:(j + 1) * D],
                            start=False, stop=True,
                        )
                        nc.scalar.activation(
                            cbt[:, i, j * D:(j + 1) * D], ps, AF.Copy
                        )
            else:
                simt, eqt = cb_mats
                for i in range(8):
                    for j in range(2):
                        ps = ps1.tile([P, D], F32, tag="pscb", name="pscb")
                        for c in range(4):
                            nc.tensor.matmul(
                                ps,
                                xnb_t[:, c, i * P:(i + 1) * P],
                                xnb_t[:, c, j * D:(j + 1) * D],
                                start=(c == 0), stop=(c == 3),
                            )
                        nc.scalar.activation(
                            simt[:, i, j * D:(j + 1) * D], ps, AF.Copy
                        )
                        ps2 = ps1.tile([P, D], F32, tag="pscb", name="pscb2")
                        nc.tensor.matmul(
                            ps2,
                            ptb_t[:, i * P:(i + 1) * P],
                            ptb_t[:, j * D:(j + 1) * D],
                            start=True, stop=True,
                        )
                        nc.scalar.activation(
                            eqt[:, i, j * D:(j + 1) * D], ps2, AF.Copy
                        )

            close("ps1", "prew", "pre")

            # ====================== PHASE 2: attention ======================
            mid = pool("mid", 1, side="right")
            attnT = mid.tile([P, 4, U], F32R, tag="attnT", name="attnT")
            den_sb = mid.tile([1, 16, D], F32, tag="densb", name="densb")

            epool = pool("epool", 3, side="right")
            pss = pool("pss", 2, space="PSUM")
            psa = pool("psa", 4, space="PSUM")

            for h in range(H):
                po = (h % 2) * 64
                ch = h // 2
                patts = [
                    psa.tile([65, D], F32, tag="psatt", name=f"psatt_{h}_{j}")
                    for j in range(2)
                ]
                for i in range(8):
                    ps = pss.tile([P, U], F32, tag="pssc", name="pssc")
                    for j in range(2):
                        mms = [(
                            kt_t[po:po + 64, ch, i * P:(i + 1) * P],
                            qt_t[po:po + 64, ch, j * D:(j + 1) * D],
                            slice(j * D, (j + 1) * D),
                        )]
                        if fast_gates:
                            adds = [(ident, cb_mats[0])]
                        else:
                            adds = [(sid_t[h], cb_mats[0]),
                                    (gid_t[h], cb_mats[1])]
                        for (idm, mat) in adds:
                            mms.append((
                                idm,
                                mat[:, i, j * D:(j + 1) * D],
                                slice(j * D, (j + 1) * D),
                            ))
                        # banded rel bias: blocks i-1, i, i+1, clipped to half j
                        lo_b = max(i - 1, 0)
                        hi_b = min(i + 1, 7)
                        run_lo = max(lo_b * P, j * D)
                        run_hi = min((hi_b + 1) * P, (j + 1) * D)
                        if run_hi > run_lo:
                            o0 = (run_lo // P) - (i - 1)
                            o1 = (run_hi // P) - (i - 1)
                            mms.append((
                                ident,
                                rb_t[:, h, o0:o1, :],
                                slice(run_lo, run_hi),
                            ))
                        for mi, (lhsT, rhs, osl) in enumerate(mms):
                            nc.tensor.matmul(
                                ps[:, osl], lhsT, rhs,
                                start=(mi == 0), stop=(mi == len(mms) - 1),
                                skip_group_check=True,
                            )
                    et = epool.tile([P, U], F32R, tag="et", name="et")
                    nc.scalar.activation(et, ps, AF.Exp)
                    for j in range(2):
                        nc.tensor.matmul(
                            patts[j],
                            vt_t[:, i, h:h + 513:8],
                            et[:, j * D:(j + 1) * D],
                            start=(i == 0), stop=(i == 7),
                        )
                for j in range(2):
                    idx = h * 2 + j
                    nc.scalar.activation(
                        den_sb[0:1, idx, :], patts[j][64:65, :], AF.Copy
                    )
                    nc.scalar.activation(
                        attnT[po:po + 64, ch, j * D:(j + 1) * D],
                        patts[j][0:64, :], AF.Copy,
                    )

            close("psa", "pss", "epool")
            close("attn_in")

            # ---------- x1 pool opens early on the left (outlives mid) -------
            x1p = pool("x1p", 1)
            x1_t = x1p.tile([P, 8, D], F32R, tag="x1", name="x1")
            x1T_t = x1p.tile([P, 4, U], F32R, tag="x1T", name="x1T")
            lnwb = x1p.tile([P, 4, D], F32, tag="lnwb", name="lnwb")
            for k in range(4):
                src = bass.AP(tensor=lnw.tensor, offset=k * D, ap=[[0, P], [1, D]])
                nc.gpsimd.dma_start(lnwb[:, k, :], src)

            # --------- denominators: gather via DRAM, recip, broadcast -------
            dn = pool("dn", 1, side="right")
            psd = pool("psd", 2, space="PSUM")
            nc.sync.dma_start(dscratch, den_sb.rearrange("o s f -> o (s f)"))
            den16 = dn.tile([16, D], F32, tag="den16", name="den16")
            nc.sync.dma_start(
                den16, dscratch.rearrange("o (p f) -> (o p) f", p=16)
            )
            rden16 = dn.tile([16, D], F32R, tag="rden16", name="rden16")
            nc.vector.reciprocal(rden16, den16)
            expd_t = dn.tile([16, 8 * P], F32R, tag="expd", name="expdt")
            nc.sync.dma_start(expd_t, expd)
            rdb = dn.tile([P, 4, U], F32, tag="rdb", name="rdb")
            for c in range(4):
                for j in range(2):
                    ps = psd.tile([P, D], F32, tag="psrd", name="psrd")
                    nc.tensor.matmul(
                        ps,
                        expd_t[:, (j * 4 + c) * P:(j * 4 + c + 1) * P],
                        rden16,
                        start=True, stop=True,
                    )
                    nc.scalar.activation(rdb[:, c, j * D:(j + 1) * D], ps, AF.Copy)
            for c in range(4):
                nc.vector.tensor_tensor(
                    attnT[:, c, :], attnT[:, c, :], rdb[:, c, :], ALU.mult
                )
            close("psd", "dn")

            # ======================= PHASE 3: X1 = LN1 ======================
            x1w = pool("x1w", 1, side="right")
            lns = pool("lns", 4, side="right")
            psc = pool("psc", 2, space="PSUM")
            pst = pool("pst", 2, space="PSUM")

            wo_t = x1w.tile([P, 4, D], F32R, tag="wo", name="wot")
            for c in range(4):
                nc.sync.dma_start(wo_t[:, c, :], wo[c])
            xpbo_t = x1w.tile([P, 8, D], F32R, tag="xpbo", name="xpbot")
            for t in range(8):
                nc.sync.dma_start(xpbo_t[:, t, :], xpbo[t])

            for t in range(8):
                ps = psc.tile([P, D], F32, tag="psx1", name="psx1")
                for c in range(4):
                    nc.tensor.matmul(
                        ps,
                        attnT[:, c, t * P:(t + 1) * P],
                        wo_t[:, c, :],
                        start=(c == 0), stop=False,
                    )
                nc.tensor.matmul(
                    ps, ident, xpbo_t[:, t, :], start=False, stop=True,
                )
                stats = lns.tile([P, 6], F32, tag="st", name="st")
                nc.vector.bn_stats(stats, ps)
                mv = lns.tile([P, 2], F32, tag="mv", name="mv")
                nc.vector.bn_aggr(mv, stats)
                rstd = lns.tile([P, 1], F32, tag="rstd", name="rstd")
                nc.scalar.activation(rstd, mv[:, 1:2], AF.Sqrt, bias=epst)
                nc.vector.reciprocal(rstd, rstd)
                xh = lns.tile([P, D], F32, tag="xh", name="xh")
                nc.vector.tensor_scalar(
                    xh, ps, mv[:, 0:1], rstd, ALU.subtract, ALU.mult
                )
                xg = lns.tile([P, D], F32, tag="xg", name="xg")
                nc.vector.tensor_tensor(xg, xh, lnwb[:, 0, :], ALU.mult)
                if apply_mask:
                    nc.vector.tensor_tensor(xg, xg, lnwb[:, 1, :], ALU.add)
                    nc.vector.tensor_scalar_mul(
                        x1_t[:, t, :], xg, valid_t[:, t:t + 1],
                    )
                else:
                    nc.vector.tensor_tensor(
                        x1_t[:, t, :], xg, lnwb[:, 1, :], ALU.add
                    )
                for c in range(4):
                    pt = pst.tile([P, P], F32R, tag="pstr", name="pstr")
                    nc.tensor.transpose(
                        pt, x1_t[:, t, c * P:(c + 1) * P], ident
                    )
                    nc.scalar.activation(
                        x1T_t[:, c, t * P:(t + 1) * P], pt, AF.Copy
                    )

            close("pst", "psc", "lns", "x1w")
            close("mid")

            # ========================= PHASE 4: FFN =========================
            ffnw = pool("ffnw", 1)  # left stack: consts, x1p, ffnw
            hidp = pool("hidp", 2, side="right")
            lns2 = pool("lns2", 4, side="right")
            outp = pool("outp", 2, side="right")
            psf = pool("psf", 4, space="PSUM")

            w1_t = ffnw.tile([P, 4, DFF], F32R, tag="w1", name="w1t")
            for c in range(4):
                nc.sync.dma_start(w1_t[:, c, :], w1[c])
            w2_t = ffnw.tile([P, 33, D], BF16, tag="w2", name="w2t")
            for c in range(33):
                nc.sync.dma_start(w2_t[:, c, :], w2[c])

            nblk = U // UBLK
            for ub in range(nblk):
                hid = hidp.tile([P, 32, UBLK], BF16, tag="hid", name="hid")
                for t in range(32):
                    ps = psf.tile([P, UBLK], F32, tag="psh", name="psh")
                    for c in range(4):
                        nc.tensor.matmul(
                            ps,
                            w1_t[:, c, t * P:(t + 1) * P],
                            x1T_t[:, c, ub * UBLK:(ub + 1) * UBLK],
                            start=(c == 0), stop=(c == 3),
                        )
                    nc.scalar.activation(
                        hid[:, t, :], ps, AF.Relu, bias=bf1_t[:, t:t + 1],
                    )
                for v in range(UBLK // P):
                    g = ub * (UBLK // P) + v
                    ps = psf.tile([P, D], F32, tag="pso", name="pso")
                    for c in range(32):
                        nc.tensor.matmul(
                            ps,
                            hid[:, c, v * P:(v + 1) * P],
                            w2_t[:, c, :],
                            start=(c == 0), stop=False,
                        )
                    nc.tensor.matmul(
                        ps, ones_bf, w2_t[0:1, 32, :],
                        start=False, stop=False, skip_group_check=True,
                    )
                    nc.tensor.matmul(
                        ps, ident, x1_t[:, g, :],
                        start=False, stop=True, skip_group_check=True,
                    )
                    stats = lns2.tile([P, 6], F32, tag="st2", name="st2")
                    nc.vector.bn_stats(stats, ps)
                    mv = lns2.tile([P, 2], F32, tag="mv2", name="mv2")
                    nc.vector.bn_aggr(mv, stats)
                    rstd = lns2.tile([P, 1], F32, tag="rstd2", name="rstd2")
                    nc.scalar.activation(rstd, mv[:, 1:2], AF.Sqrt, bias=epst)
                    nc.vector.reciprocal(rstd, rstd)
                    xh = lns2.tile([P, D], F32, tag="xh2", name="xh2")
                    nc.vector.tensor_scalar(
                        xh, ps, mv[:, 0:1], rstd, ALU.subtract, ALU.mult
                    )
                    xg = lns2.tile([P, D], F32, tag="xg2", name="xg2")
                    nc.vector.tensor_tensor(xg, xh, lnwb[:, 2, :], ALU.mult)
                    x2 = outp.tile([P, D], F32, tag="x2", name="x2")
                    if apply_mask:
                        nc.vector.tensor_tensor(xg, xg, lnwb[:, 3, :], ALU.add)
                        nc.vector.tensor_scalar_mul(
                            x2, xg, valid_t[:, g:g + 1]
                        )
                    else:
                        nc.vector.tensor_tensor(x2, xg, lnwb[:, 3, :], ALU.add)
                    nc.sync.dma_start(out[g], x2)

            close("psf", "outp", "lns2", "hidp", "ffnw", "x1p", "consts")
        finally:
            for n in list(open_cms):
                try:
                    open_cms.pop(n).__exit__(None, None, None)
                except Exception:
                    pass

    nc.compile()
    return nc


def _get_program(fast_gates, apply_mask, ncat):
    key = (fast_gates, apply_mask, ncat)
    if key not in _prog_cache:
        _prog_cache[key] = _build_program(fast_gates, apply_mask, ncat)
    return _prog_cache[key]


def kernel(**inputs):
    X = np.ascontiguousarray(np.asarray(inputs["X"], dtype=np.float32))
    mask = np.asarray(inputs["mask_u"]).astype(bool)
    spk = np.asarray(inputs["speakers"]).astype(np.int64)
    Wq = np.asarray(inputs["Wq"], np.float32); bq = np.asarray(inputs["bq"], np.float32)
    Wk = np.asarray(inputs["Wk"], np.float32); bk = np.asarray(inputs["bk"], np.float32)
    Wv = np.asarray(inputs["Wv"], np.float32); bv = np.asarray(inputs["bv"], np.float32)
    Wo = np.asarray(inputs["Wo"], np.float32); bo = np.asarray(inputs["bo"], np.float32)
    relb = np.asarray(inputs["rel_bias"], np.float32)
    gate = np.asarray(inputs["speaker_gate"], np.float32)
    sims = np.asarray(inputs["sim_scale"], np.float32)
    g1 = np.asarray(inputs["g1"], np.float32); beta1 = np.asarray(inputs["beta1"], np.float32)
    g2 = np.asarray(inputs["g2"], np.float32); beta2 = np.asarray(inputs["beta2"], np.float32)
    W1 = np.asarray(inputs["W1"], np.float32); bf1 = np.asarray(inputs["bf1"], np.float32)
    W2 = np.asarray(inputs["W2"], np.float32); bf2 = np.asarray(inputs["bf2"], np.float32)

    ncat = int(max(9, spk.max() + 1))
    fast_gates = bool(np.all(gate == gate[0]) and np.all(sims == sims[0]))
    apply_mask = not bool(mask.all())

    nc = _get_program(fast_gates, apply_mask, ncat)

    # ---- shared (weight) arrays ----
    scale = 1.0 / math.sqrt(DH)
    wq_a = np.ascontiguousarray((Wq * scale).reshape(4, P, D))
    wk_a = np.ascontiguousarray(Wk.reshape(4, P, D))
    wv_a = np.concatenate([Wv.reshape(4, P, D), np.zeros((1, P, D), np.float32)], 0)
    wv_a[4, 0, :] = bv
    wv_a = np.ascontiguousarray(wv_a)
    wo_a = np.ascontiguousarray(Wo.reshape(4, P, D))
    w1_a = np.ascontiguousarray(W1.reshape(4, P, DFF))
    w2_a = np.concatenate([W2.reshape(32, P, D), np.zeros((1, P, D), np.float32)], 0)
    w2_a[32, 0, :] = bf2
    w2_a = np.ascontiguousarray(w2_a.astype(ml_dtypes.bfloat16))
    bf1p_a = np.ascontiguousarray(bf1.reshape(32, P).T)
    qkb_a = np.zeros((P, 8), np.float32)
    qkb_a[:, 0:4] = (bq * scale).reshape(4, P).T
    qkb_a[:, 4:8] = bk.reshape(4, P).T
    lnw_a = np.ascontiguousarray(np.stack([g1, beta1, g2, beta2]))

    # banded rel bias: rb[a, h, o, c] = relb[h, min(|(o-1)*128+c-a|,128)] - relb[h,128]
    a_i = np.arange(P)[:, None]
    c_i = np.arange(P)[None, :]
    rb_hoc = np.zeros((H, 3, P, P), np.float32)
    for o in range(3):
        dist = np.minimum(np.abs((o - 1) * P + c_i - a_i), REL_MAX)
        rb_hoc[:, o] = relb[:, dist] - relb[:, REL_MAX][:, None, None]
    rbd_a = np.ascontiguousarray(rb_hoc.transpose(2, 0, 1, 3))  # [a, h, o, c]

    # denominator-broadcast expander
    expd_a = np.zeros((16, 2, 4, P), np.float32)
    for j in range(2):
        for c in range(4):
            expd_a[4 * c + j, j, c, 0:64] = 1.0
            expd_a[4 * c + 2 + j, j, c, 64:P] = 1.0
    expd_a = np.ascontiguousarray(expd_a.reshape(16, 8 * P))

    ident_a = np.eye(P, dtype=np.float32)

    shared = dict(wq=wq_a, wk=wk_a, wv=wv_a, wo=wo_a, w1=w1_a, w2=w2_a,
                  bf1p=bf1p_a, qkb=qkb_a, lnw=lnw_a, rbd=rbd_a, expd=expd_a,
                  identd=ident_a,
                  ones_pe=np.ones((1, P), np.float32),
                  ones_pb=np.ones((1, P), ml_dtypes.bfloat16),
                  ones_v=np.ones((P, 64), np.float32))
    if not fast_gates:
        shared["sidents"] = np.ascontiguousarray(sims[:, None, None] * ident_a[None])
        shared["gidents"] = np.ascontiguousarray(-gate[:, None, None] * ident_a[None])

    in_maps = []
    for b in range(B):
        Xb = X[b]
        validf = mask[b].astype(np.float32)
        norm = np.linalg.norm(Xb, axis=-1)
        rn = (1.0 / np.maximum(norm, 1e-6)) * validf
        Pmat = np.zeros((U, ncat), np.float32)
        Pmat[np.arange(U), np.clip(spk[b], 0, ncat - 1)] = 1.0
        ptb_a = np.ascontiguousarray(Pmat.T)
        pta_a = np.ascontiguousarray((-gate)[:, None, None] * ptb_a[None])
        m = dict(
            xt=np.ascontiguousarray(Xb.T).reshape(4, P, U),
            xpbo=np.ascontiguousarray((Xb + bo).reshape(8, P, D)),
            rns_a=np.ascontiguousarray((sims[0] * rn)[None, :]),
            rns_b=np.ascontiguousarray(rn[None, :]),
            pta=pta_a,
            ptb=ptb_a,
            validd=np.ascontiguousarray(validf.reshape(8, P).T),
            **shared,
        )
        in_maps.append(m)

    res = run_bass_kernel_spmd(nc, in_maps, core_ids=list(range(NCORES)))
    outs = [r["out"].reshape(U, D) for r in res.results]
    return np.stack(outs).astype(np.float32)


# revision 25
# speedup vs baseline: 6.9457x; 6.9457x over previous
"""Trainium2 Bass kernel for nn_BiasedMHABlock (biased MHA + FFN transformer block).

Sharding: batch B=8 -> one batch per NeuronCore (SPMD, no collectives).

Per-core math (batch b), fully fused on-device:
  scores^T[w,u] per head = (K_h Q_h^T)/8 + CB[w,u] + relband_h[w,u], where
  softmax-invariant constants are dropped and
  CB = simscale*Xn Xn^T - gate*OneHot(spk) OneHot(spk)^T is built once via PE
  and added per-head with identity-matmul PSUM accumulation.
  Softmax runs over the partition axis without max-subtraction (scores are O(1));
  the denominator comes free from an appended ones-column of V in the attn@V
  matmul and is divided out post-hoc.
  Then X1 = LN(X+bo + Attn@Wo), FFN with transposed hidden, X2 = LN(X1+ffn).

All matmuls run as float32r (full-rate fp32); ffn2 runs bf16.
"""
import sys
import math

for _p in ("/opt/trn_rl_repo",):
    if _p not in sys.path:
        sys.path.insert(0, _p)

import numpy as np
import ml_dtypes

import concourse.bass as bass
import concourse.tile as tile
from concourse import bacc, mybir
from concourse.bass_utils import run_bass_kernel_spmd

F32 = mybir.dt.float32
F32R = mybir.dt.float32r
BF16 = mybir.dt.bfloat16
AF = mybir.ActivationFunctionType
ALU = mybir.AluOpType

B, U, D, H, DH, DFF = 8, 1024, 512, 8, 64, 4096
REL_MAX = 128
P = 128
NCORES = 8
LN_EPS = 1e-5
UBLK = 512  # ffn u-block

_prog_cache = {}
SKIP = set()  # perf-analysis only: phase names to skip


def _build_program(fast_gates: bool, apply_mask: bool, ncat: int, ln1_triv: bool = False, ln2_triv: bool = False):
    nc = bacc.Bacc("TRN2", target_bir_lowering=False, debug=False)

    def din(name, shape, dt=F32R):
        return nc.dram_tensor(name, list(shape), dt, kind="ExternalInput").ap()

    xt = din("xt", [4, P, U])
    xpbo = din("xpbo", [8, P, D], F32)
    rns_a = din("rns_a", [1, U], F32)
    rns_b = din("rns_b", [1, U], F32)
    pta = din("pta", [H, ncat, U])
    ptb = din("ptb", [ncat, U])
    wq = din("wq", [4, P, D])
    wk = din("wk", [4, P, D])
    wv = din("wv", [5, P, D])
    wo = din("wo", [4, P, D])
    w1 = din("w1", [4, P, DFF])
    w2 = din("w2", [33, P, D])
    bf1p = din("bf1p", [P, 32], F32)
    qkb = din("qkb", [P, 8], F32)
    rbd = din("rbd", [P, H, 3, P])
    lnw = din("lnw", [4, D], F32)
    expd = din("expd", [4, 2 * P])
    uvec4 = din("uvec4", [1, 16])
    identd = din("identd", [P, P])
    identfd = din("identfd", [P, P], F32)
    ones_pe = din("ones_pe", [1, P])
    ones_v = din("ones_v", [P, 64])
    validd = din("validd", [P, 8], F32)
    if not fast_gates:
        sidents = din("sidents", [H, P, P])
        gidents = din("gidents", [H, P, P])

    out = nc.dram_tensor("out", [8, P, D], F32, kind="ExternalOutput").ap()

    open_cms = {}

    with tile.TileContext(nc) as tc, nc.allow_low_precision(reason="fp32r kernel"):
        def pool(name, bufs, space="SBUF", side="left"):
            cm = tc.tile_pool(name=name, bufs=bufs, space=space, side=side)
            p = cm.__enter__()
            open_cms[name] = cm
            return p

        def close(*names):
            for n in names:
                open_cms.pop(n).__exit__(None, None, None)

        try:
            # ---------------- constants (left, whole-kernel) ----------------
            consts = pool("consts", 1)
            ident = consts.tile([P, P], F32R)
            identf = consts.tile([P, P], F32)
            qkb_t = consts.tile([P, 8], F32)
            bf1_t = consts.tile([P, 32], F32)
            valid_t = consts.tile([P, 8], F32)
            epst = consts.tile([P, 1], F32)
            ones_u = consts.tile([1, P], F32R)
            ones_bf = consts.tile([1, P], F32R)

            # ------------- long-lived attention inputs (left) ----------------
            attn_in = pool("attn_in", 1)
            qt_t = attn_in.tile([P, 4, U], F32R, tag="qt", name="qt")
            kt_t = attn_in.tile([P, 4, U], F32R, tag="kt", name="kt")
            vt_t = attn_in.tile([P, 8, 520], F32R, tag="vt", name="vt")
            rb_t = attn_in.tile([P, H, 3, P], F32R, tag="rbt", name="rbt")
            if fast_gates:
                cb_mats = [attn_in.tile([P, 8, U], F32R, tag="cbt", name="cbt")]
            else:
                cb_mats = [
                    attn_in.tile([P, 8, U], F32R, tag="simt", name="simt"),
                    attn_in.tile([P, 8, U], F32R, tag="eqt", name="eqt"),
                ]
                sid_t = attn_in.tile([P, H, P], F32R, tag="sid", name="sid")
                nc.sync.dma_start(sid_t, sidents.rearrange("h p q -> p h q"))
                gid_t = attn_in.tile([P, H, P], F32R, tag="gid", name="gid")
                nc.sync.dma_start(gid_t, gidents.rearrange("h p q -> p h q"))

            # ======================= PHASE 1: prep ==========================
            pre = pool("pre", 1, side="right")
            prew = pool("prew", 2, side="right")
            ps1 = pool("ps1", 2, space="PSUM")

            xt_t = pre.tile([P, 4, U], F32R, tag="xt", name="xtt")
            nc.sync.dma_start(xt_t[:, 0, 0:D], xt[0][:, 0:D])
            if fast_gates:
                rnsa_b = pre.tile([P, U], F32, tag="rnsa", name="rnsa")
                nc.gpsimd.dma_start(
                    rnsa_b,
                    bass.AP(tensor=rns_a.tensor, offset=0, ap=[[0, P], [1, U]]),
                )
            rnsb_b = pre.tile([P, U], F32, tag="rnsb", name="rnsb")
            nc.gpsimd.dma_start(
                rnsb_b, bass.AP(tensor=rns_b.tensor, offset=0, ap=[[0, P], [1, U]])
            )

            wq_t = prew.tile([P, 5, D], F32R, tag="wx", name="wqt")
            wk_t = prew.tile([P, 5, D], F32R, tag="wx", name="wkt")
            nc.sync.dma_start(wq_t[:, 0, :], wq[0])
            for c in range(1, 4):
                nc.sync.dma_start(xt_t[:, c, 0:D], xt[c][:, 0:D])
                nc.sync.dma_start(wq_t[:, c, :], wq[c])
            for c in range(4):
                nc.sync.dma_start(xt_t[:, c, D:U], xt[c][:, D:U])
                nc.sync.dma_start(wk_t[:, c, :], wk[c])
            nc.sync.dma_start(qkb_t, qkb)
            nc.sync.dma_start(ident, identd)
            nc.sync.dma_start(identf, identfd)
            nc.sync.dma_start(bf1_t, bf1p)
            nc.sync.dma_start(valid_t, validd)
            nc.vector.memset(epst, LN_EPS)
            nc.sync.dma_start(ones_u, ones_pe)
            nc.sync.dma_start(ones_bf, ones_pe)

            # Q^T, K^T: psum[e_tile, u_half] = sum_c Wx[c]-slice^T @ XT
            for (wt, dst, boff) in () if "qkproj" in SKIP else ((wq_t, qt_t, 0), (wk_t, kt_t, 4)):
                for t in range(4):
                    for j in range(2):
                        ps = ps1.tile([P, D], F32, tag="psqk", name="psqk")
                        for c in range(4):
                            nc.tensor.matmul(
                                ps,
                                wt[:, c, t * P:(t + 1) * P],
                                xt_t[:, c, j * D:(j + 1) * D],
                                start=(c == 0), stop=(c == 3),
                            )
                        nc.scalar.activation(
                            dst[:, t, j * D:(j + 1) * D], ps, AF.Identity,
                            bias=qkb_t[:, boff + t:boff + t + 1],
                        )

            # V (interleaved (dh h) layout + ones cols)
            wv_t = prew.tile([P, 5, D], F32R, tag="wx", name="wvt")
            for c in range(5):
                nc.sync.dma_start(wv_t[:, c, :], wv[c])
            for t in range(0 if "vproj" in SKIP else 8):
                ps = ps1.tile([P, D], F32, tag="psv", name="psv")
                for c in range(4):
                    nc.tensor.matmul(
                        ps, xt_t[:, c, t * P:(t + 1) * P], wv_t[:, c, :],
                        start=(c == 0), stop=False,
                    )
                nc.tensor.matmul(
                    ps, ones_u[0:1, :], wv_t[0:1, 4, :],
                    start=False, stop=True,
                )
                nc.vector.tensor_copy(
                    vt_t[:, t, 0:D].rearrange("p (dh h) -> p dh h", h=H),
                    ps.rearrange("p (h dh) -> p dh h", h=H),
                )
            nc.sync.dma_start(
                vt_t[:, :, D:520], ones_v.rearrange("p (t f) -> p t f", t=8)
            )
            if apply_mask:
                for t in range(8):
                    nc.vector.tensor_scalar_mul(
                        vt_t[:, t, :], vt_t[:, t, :], valid_t[:, t:t + 1],
                    )

            # Xn^T (scaled / unscaled sides) and CB (or SIM + EQ)
            if fast_gates:
                xna_t = pre.tile([P, 4, U], F32R, tag="xna", name="xna")
            xnb_t = pre.tile([P, 4, U], F32R, tag="xnb", name="xnb")
            for c in range(4):
                if fast_gates:
                    nc.vector.tensor_tensor(
                        xna_t[:, c, :], xt_t[:, c, :], rnsa_b, ALU.mult
                    )
                nc.vector.tensor_tensor(
                    xnb_t[:, c, :], xt_t[:, c, :], rnsb_b, ALU.mult
                )

            ptb_t = pre.tile([ncat, U], F32R, tag="ptb", name="ptbt")
            nc.sync.dma_start(ptb_t, ptb)
            if fast_gates:
                pta_t = pre.tile([ncat, 1, U], F32R, tag="pta", name="ptat")
                nc.sync.dma_start(pta_t[:, 0, :], pta[0])

            if fast_gates:
                cbt = cb_mats[0]
                for i in range(0 if "cb" in SKIP else 8):
                    for j in range(2):
                        ps = ps1.tile([P, D], F32, tag="pscb", name="pscb")
                        for c in range(4):
                            nc.tensor.matmul(
                                ps,
                                xna_t[:, c, i * P:(i + 1) * P],
                                xnb_t[:, c, j * D:(j + 1) * D],
                                start=(c == 0), stop=False,
                            )
                        nc.tensor.matmul(
                            ps,
                            pta_t[:, 0, i * P:(i + 1) * P],
                            ptb_t[:, j * D:(j + 1) * D],
                            start=False, stop=True,
                        )
                        nc.vector.tensor_copy(
                            cbt[:, i, j * D:(j + 1) * D], ps
                        )
            else:
                simt, eqt = cb_mats
                for i in range(8):
                    for j in range(2):
                        ps = ps1.tile([P, D], F32, tag="pscb", name="pscb")
                        for c in range(4):
                            nc.tensor.matmul(
                                ps,
                                xnb_t[:, c, i * P:(i + 1) * P],
                                xnb_t[:, c, j * D:(j + 1) * D],
                                start=(c == 0), stop=(c == 3),
                            )
                        nc.scalar.activation(
                            simt[:, i, j * D:(j + 1) * D], ps, AF.Copy
                        )
                        ps2 = ps1.tile([P, D], F32, tag="pscb", name="pscb2")
                        nc.tensor.matmul(
                            ps2,
                            ptb_t[:, i * P:(i + 1) * P],
                            ptb_t[:, j * D:(j + 1) * D],
                            start=True, stop=True,
                        )
                        nc.scalar.activation(
                            eqt[:, i, j * D:(j + 1) * D], ps2, AF.Copy
                        )

            close("ps1", "prew", "pre")

            nc.sync.dma_start(rb_t, rbd)

            # ====================== PHASE 2: attention ======================
            mid = pool("mid", 1, side="right")
            attnT = mid.tile([P, 4, U], F32R, tag="attnT", name="attnT")
            den_sb = mid.tile([1, 16, D], F32R, tag="densb", name="densb")
            expd_t = mid.tile([4, 2 * P], F32R, tag="expd", name="expdt")
            nc.sync.dma_start(expd_t, expd)
            uvec_t = mid.tile([1, 16], F32R, tag="uvec", name="uvect")
            nc.sync.dma_start(uvec_t, uvec4)

            epool = pool("epool", 3, side="right")
            dnp = pool("dnp", 4, side="right")
            pss = pool("pss", 4, space="PSUM")
            psa = pool("psa", 4, space="PSUM")

            for h in range(0 if "attn" in SKIP else H):
                po = (h % 2) * 64
                ch = h // 2
                patts = [
                    psa.tile([65, D], F32, tag="psatt", name=f"psatt_{h}_{j}")
                    for j in range(2)
                ]
                for i in range(8):
                    et = epool.tile([P, U], F32R, tag="et", name="et")
                    for j in range(2):
                        ps = pss.tile([P, D], F32, tag="pssc", name="pssc")
                        mms = [(
                            kt_t[po:po + 64, ch, i * P:(i + 1) * P],
                            qt_t[po:po + 64, ch, j * D:(j + 1) * D],
                            slice(0, D),
                        )]
                        if fast_gates:
                            adds = [(ident, cb_mats[0])]
                        else:
                            adds = [(sid_t[:, h, :], cb_mats[0]),
                                    (gid_t[:, h, :], cb_mats[1])]
                        for (idm, mat) in adds:
                            mms.append((
                                idm,
                                mat[:, i, j * D:(j + 1) * D],
                                slice(0, D),
                            ))
                        # banded rel bias: blocks i-1, i, i+1, clipped to half j
                        lo_b = max(i - 1, 0)
                        hi_b = min(i + 1, 7)
                        run_lo = max(lo_b * P, j * D)
                        run_hi = min((hi_b + 1) * P, (j + 1) * D)
                        if run_hi > run_lo:
                            o0 = (run_lo // P) - (i - 1)
                            o1 = (run_hi // P) - (i - 1)
                            mms.append((
                                ident,
                                rb_t[:, h, o0:o1, :],
                                slice(run_lo - j * D, run_hi - j * D),
                            ))
                        for mi, (lhsT, rhs, osl) in enumerate(mms):
                            nc.tensor.matmul(
                                ps[:, osl], lhsT, rhs,
                                start=(mi == 0), stop=(mi == len(mms) - 1),
                                skip_group_check=True,
                            )
                        nc.scalar.activation(
                            et[:, j * D:(j + 1) * D], ps, AF.Exp
                        )
                    for j in range(2):
                        nc.tensor.matmul(
                            patts[j],
                            vt_t[:, i, h:h + 513:8],
                            et[:, j * D:(j + 1) * D],
                            start=(i == 0), stop=(i == 7),
                        )
                for j in range(2):
                    idx = h * 2 + j
                    nc.vector.tensor_copy(
                        den_sb[0:1, idx, :], patts[j][64:65, :]
                    )
                    nc.vector.tensor_copy(
                        attnT[po:po + 64, ch, j * D:(j + 1) * D],
                        patts[j][0:64, :],
                    )
                if h % 2 == 1:
                    # normalize chunk ch: heads 2ch, 2ch+1 are done
                    c4 = 4 * ch
                    psg = psa.tile([4, D], F32, tag="psatt", name=f"psg_{ch}")
                    for r in range(4):
                        nc.tensor.matmul(
                            psg,
                            uvec_t[0:1, r * 4:(r + 1) * 4],
                            den_sb[0:1, c4 + r, :],
                            start=(r == 0), stop=(r == 3),
                        )
                    rden4 = dnp.tile([4, D], F32R, tag="rden4", name="rden4")
                    nc.vector.reciprocal(rden4, psg)
                    for j in range(2):
                        psn = psa.tile([P, D], F32, tag="psatt", name=f"psn_{ch}_{j}")
                        nc.tensor.matmul(
                            psn,
                            expd_t[:, j * P:(j + 1) * P],
                            rden4,
                            start=True, stop=True,
                        )
                        nc.vector.tensor_tensor(
                            attnT[:, ch, j * D:(j + 1) * D],
                            attnT[:, ch, j * D:(j + 1) * D],
                            psn, ALU.mult,
                        )

            close("psa", "pss", "dnp", "epool")
            close("attn_in")

            # ---------- x1 pool opens early on the left (outlives mid) -------
            x1p = pool("x1p", 1)
            x1_t = x1p.tile([P, 8, D], F32, tag="x1", name="x1")
            x1T_t = x1p.tile([P, 4, U], F32R, tag="x1T", name="x1T")
            lnwb = None
            if not (ln1_triv and ln2_triv):
                lnwb = x1p.tile([P, 4, D], F32, tag="lnwb", name="lnwb")
                for k in range(4):
                    src = bass.AP(tensor=lnw.tensor, offset=k * D,
                                  ap=[[0, P], [1, D]])
                    nc.gpsimd.dma_start(lnwb[:, k, :], src)


            # ======================= PHASE 3: X1 = LN1 ======================
            x1w = pool("x1w", 1, side="right")
            lns = pool("lns", 4, side="right")
            psc = pool("psc", 3, space="PSUM")
            pst = pool("pst", 3, space="PSUM")

            wo_t = x1w.tile([P, 4, D], F32R, tag="wo", name="wot")
            for c in range(4):
                nc.sync.dma_start(wo_t[:, c, :], wo[c])
            xpbo_t = x1w.tile([P, 8, D], F32, tag="xpbo", name="xpbot")
            for t in range(8):
                nc.sync.dma_start(xpbo_t[:, t, :], xpbo[t])

            for t in range(0 if "x1" in SKIP else 8):
                ps = psc.tile([P, D], F32, tag="psx1", name="psx1")
                for c in range(4):
                    nc.tensor.matmul(
                        ps,
                        attnT[:, c, t * P:(t + 1) * P],
                        wo_t[:, c, :],
                        start=(c == 0), stop=(c == 3),
                    )
                o1 = lns.tile([P, D], F32, tag="o1", name="o1")
                nc.vector.tensor_tensor(o1, ps, xpbo_t[:, t, :], ALU.add)
                ps = o1
                stats = lns.tile([P, 6], F32, tag="st", name="st")
                nc.vector.bn_stats(stats, ps)
                mv = lns.tile([P, 2], F32, tag="mv", name="mv")
                nc.vector.bn_aggr(mv, stats)
                rstd = lns.tile([P, 1], F32, tag="rstd", name="rstd")
                nc.scalar.activation(rstd, mv[:, 1:2], AF.Sqrt, bias=epst)
                nc.vector.reciprocal(rstd, rstd)
                if ln1_triv and not apply_mask:
                    nc.vector.tensor_scalar(
                        x1_t[:, t, :], ps, mv[:, 0:1], rstd,
                        ALU.subtract, ALU.mult,
                    )
                elif ln1_triv:
                    xh = lns.tile([P, D], F32, tag="xh", name="xh")
                    nc.vector.tensor_scalar(
                        xh, ps, mv[:, 0:1], rstd, ALU.subtract, ALU.mult
                    )
                    nc.vector.tensor_scalar_mul(
                        x1_t[:, t, :], xh, valid_t[:, t:t + 1],
                    )
                else:
                    xh = lns.tile([P, D], F32, tag="xh", name="xh")
                    nc.vector.tensor_scalar(
                        xh, ps, mv[:, 0:1], rstd, ALU.subtract, ALU.mult
                    )
                    xg = lns.tile([P, D], F32, tag="xg", name="xg")
                    nc.vector.tensor_tensor(xg, xh, lnwb[:, 0, :], ALU.mult)
                    if apply_mask:
                        nc.vector.tensor_tensor(xg, xg, lnwb[:, 1, :], ALU.add)
                        nc.vector.tensor_scalar_mul(
                            x1_t[:, t, :], xg, valid_t[:, t:t + 1],
                        )
                    else:
                        nc.vector.tensor_tensor(
                            x1_t[:, t, :], xg, lnwb[:, 1, :], ALU.add
                        )
                for c in range(4):
                    pt = pst.tile([P, P], F32, tag="pstr", name="pstr")
                    nc.tensor.transpose(
                        pt, x1_t[:, t, c * P:(c + 1) * P], identf
                    )
                    nc.scalar.activation(
                        x1T_t[:, c, t * P:(t + 1) * P], pt, AF.Copy
                    )

            close("pst", "psc", "lns", "x1w")
            close("mid")

            # ========================= PHASE 4: FFN =========================
            ffnw = pool("ffnw", 1)  # left stack: consts, x1p, ffnw
            hidp = pool("hidp", 1, side="right")
            w2s = pool("w2s", 6, side="right")
            lns2 = pool("lns2", 4, side="right")
            outp = pool("outp", 2, side="right")
            psf = pool("psf", 4, space="PSUM")

            w1_t = ffnw.tile([P, 4, DFF], F32R, tag="w1", name="w1t")
            for c in range(4):
                nc.sync.dma_start(w1_t[:, c, :], w1[c])

            ublk = UBLK if (ln1_triv and ln2_triv and not apply_mask) else 256
            nblk = 0 if "ffn" in SKIP else U // ublk
            for ub in range(nblk):
                hid = hidp.tile([P, 32, ublk], F32R, tag="hid", name="hid")
                for t in range(32):
                    ps = psf.tile([P, ublk], F32, tag="psh", name="psh")
                    for c in range(4):
                        nc.tensor.matmul(
                            ps,
                            w1_t[:, c, t * P:(t + 1) * P],
                            x1T_t[:, c, ub * ublk:(ub + 1) * ublk],
                            start=(c == 0), stop=(c == 3),
                        )
                    nc.scalar.activation(
                        hid[:, t, :], ps, AF.Relu, bias=bf1_t[:, t:t + 1],
                    )
                nv = ublk // P
                psos = [
                    psf.tile([P, D], F32, tag="pso", name=f"pso{v}")
                    for v in range(nv)
                ]
                for c in range(33):
                    w2c = w2s.tile([P, D], F32R, tag="w2c", name="w2c")
                    nc.sync.dma_start(w2c, w2[c])
                    for v in range(nv):
                        if c < 32:
                            nc.tensor.matmul(
                                psos[v],
                                hid[:, c, v * P:(v + 1) * P],
                                w2c,
                                start=(c == 0), stop=False,
                                skip_group_check=True,
                            )
                        else:
                            nc.tensor.matmul(
                                psos[v], ones_bf, w2c[0:1, :],
                                start=False, stop=True, skip_group_check=True,
                            )
                for v in range(nv):
                    g = ub * nv + v
                    ps = psos[v]
                    x2p = lns2.tile([P, D], F32, tag="x2p", name="x2p")
                    nc.vector.tensor_tensor(x2p, ps, x1_t[:, g, :], ALU.add)
                    ps = x2p
                    stats = lns2.tile([P, 6], F32, tag="st2", name="st2")
                    nc.vector.bn_stats(stats, ps)
                    mv = lns2.tile([P, 2], F32, tag="mv2", name="mv2")
                    nc.vector.bn_aggr(mv, stats)
                    rstd = lns2.tile([P, 1], F32, tag="rstd2", name="rstd2")
                    nc.scalar.activation(rstd, mv[:, 1:2], AF.Sqrt, bias=epst)
                    nc.vector.reciprocal(rstd, rstd)
                    x2 = outp.tile([P, D], F32, tag="x2", name="x2")
                    if ln2_triv and not apply_mask:
                        nc.vector.tensor_scalar(
                            x2, ps, mv[:, 0:1], rstd, ALU.subtract, ALU.mult
                        )
                        nc.sync.dma_start(out[g], x2)
                    elif ln2_triv:
                        xh = lns2.tile([P, D], F32, tag="xh2", name="xh2")
                        nc.vector.tensor_scalar(
                            xh, ps, mv[:, 0:1], rstd, ALU.subtract, ALU.mult
                        )
                        nc.vector.tensor_scalar_mul(x2, xh, valid_t[:, g:g + 1])
                        nc.sync.dma_start(out[g], x2)
                    else:
                        xh = lns2.tile([P, D], F32, tag="xh2", name="xh2")
                        nc.vector.tensor_scalar(
                            xh, ps, mv[:, 0:1], rstd, ALU.subtract, ALU.mult
                        )
                        xg = lns2.tile([P, D], F32, tag="xg2", name="xg2")
                        nc.vector.tensor_tensor(xg, xh, lnwb[:, 2, :], ALU.mult)
                        if apply_mask:
                            nc.vector.tensor_tensor(xg, xg, lnwb[:, 3, :], ALU.add)
                            nc.vector.tensor_scalar_mul(
                                x2, xg, valid_t[:, g:g + 1]
                            )
                        else:
                            nc.vector.tensor_tensor(x2, xg, lnwb[:, 3, :], ALU.add)
                        nc.sync.dma_start(out[g], x2)

            close("psf", "outp", "lns2", "w2s", "hidp", "ffnw", "x1p", "consts")
        finally:
            for n in list(open_cms):
                try:
                    open_cms.pop(n).__exit__(None, None, None)
                except Exception:
                    pass

    nc.compile()
    return nc


def _get_program(fast_gates, apply_mask, ncat, ln1_triv=False, ln2_triv=False):
    key = (fast_gates, apply_mask, ncat, ln1_triv, ln2_triv)
    if key not in _prog_cache:
        _prog_cache[key] = _build_program(fast_gates, apply_mask, ncat,
                                          ln1_triv, ln2_triv)
    return _prog_cache[key]


def kernel(**inputs):
    X = np.ascontiguousarray(np.asarray(inputs["X"], dtype=np.float32))
    mask = np.asarray(inputs["mask_u"]).astype(bool)
    spk = np.asarray(inputs["speakers"]).astype(np.int64)
    Wq = np.asarray(inputs["Wq"], np.float32); bq = np.asarray(inputs["bq"], np.float32)
    Wk = np.asarray(inputs["Wk"], np.float32); bk = np.asarray(inputs["bk"], np.float32)
    Wv = np.asarray(inputs["Wv"], np.float32); bv = np.asarray(inputs["bv"], np.float32)
    Wo = np.asarray(inputs["Wo"], np.float32); bo = np.asarray(inputs["bo"], np.float32)
    relb = np.asarray(inputs["rel_bias"], np.float32)
    gate = np.asarray(inputs["speaker_gate"], np.float32)
    sims = np.asarray(inputs["sim_scale"], np.float32)
    g1 = np.asarray(inputs["g1"], np.float32); beta1 = np.asarray(inputs["beta1"], np.float32)
    g2 = np.asarray(inputs["g2"], np.float32); beta2 = np.asarray(inputs["beta2"], np.float32)
    W1 = np.asarray(inputs["W1"], np.float32); bf1 = np.asarray(inputs["bf1"], np.float32)
    W2 = np.asarray(inputs["W2"], np.float32); bf2 = np.asarray(inputs["bf2"], np.float32)

    ncat = int(max(9, spk.max() + 1))
    fast_gates = bool(np.all(gate == gate[0]) and np.all(sims == sims[0]))
    apply_mask = not bool(mask.all())

    ln1_triv = bool(np.all(g1 == 1.0) and np.all(beta1 == 0.0))
    ln2_triv = bool(np.all(g2 == 1.0) and np.all(beta2 == 0.0))
    nc = _get_program(fast_gates, apply_mask, ncat, ln1_triv, ln2_triv)

    # ---- shared (weight) arrays ----
    scale = 1.0 / math.sqrt(DH)
    wq_a = np.ascontiguousarray((Wq * scale).reshape(4, P, D))
    wk_a = np.ascontiguousarray(Wk.reshape(4, P, D))
    wv_a = np.concatenate([Wv.reshape(4, P, D), np.zeros((1, P, D), np.float32)], 0)
    wv_a[4, 0, :] = bv
    wv_a = np.ascontiguousarray(wv_a)
    wo_a = np.ascontiguousarray(Wo.reshape(4, P, D))
    w1_a = np.ascontiguousarray(W1.reshape(4, P, DFF))
    w2_a = np.concatenate([W2.reshape(32, P, D), np.zeros((1, P, D), np.float32)], 0)
    w2_a[32, 0, :] = bf2
    w2_a = np.ascontiguousarray(w2_a)
    bf1p_a = np.ascontiguousarray(bf1.reshape(32, P).T)
    qkb_a = np.zeros((P, 8), np.float32)
    qkb_a[:, 0:4] = (bq * scale).reshape(4, P).T
    qkb_a[:, 4:8] = bk.reshape(4, P).T
    lnw_a = np.ascontiguousarray(np.stack([g1, beta1, g2, beta2]))

    # banded rel bias: rb[a, h, o, c] = relb[h, min(|(o-1)*128+c-a|,128)] - relb[h,128]
    a_i = np.arange(P)[:, None]
    c_i = np.arange(P)[None, :]
    rb_hoc = np.zeros((H, 3, P, P), np.float32)
    for o in range(3):
        dist = np.minimum(np.abs((o - 1) * P + c_i - a_i), REL_MAX)
        rb_hoc[:, o] = relb[:, dist] - relb[:, REL_MAX][:, None, None]
    rbd_a = np.ascontiguousarray(rb_hoc.transpose(2, 0, 1, 3))  # [a, h, o, c]

    # denominator-broadcast expander: r = (h - 2c)*2 + j
    expd_a = np.zeros((4, 2, P), np.float32)
    for j in range(2):
        expd_a[j, j, 0:64] = 1.0
        expd_a[2 + j, j, 64:P] = 1.0
    expd_a = np.ascontiguousarray(expd_a.reshape(4, 2 * P))

    ident_a = np.eye(P, dtype=np.float32)
    uvec4_a = np.ascontiguousarray(np.eye(4, dtype=np.float32).reshape(1, 16))

    shared = dict(wq=wq_a, wk=wk_a, wv=wv_a, wo=wo_a, w1=w1_a, w2=w2_a,
                  bf1p=bf1p_a, qkb=qkb_a, lnw=lnw_a, rbd=rbd_a, expd=expd_a,
                  identd=ident_a, identfd=ident_a, uvec4=uvec4_a,
                  ones_pe=np.ones((1, P), np.float32),
                  ones_v=np.ones((P, 64), np.float32))
    if not fast_gates:
        shared["sidents"] = np.ascontiguousarray(sims[:, None, None] * ident_a[None])
        shared["gidents"] = np.ascontiguousarray(-gate[:, None, None] * ident_a[None])

    in_maps = []
    for b in range(B):
        Xb = X[b]
        validf = mask[b].astype(np.float32)
        norm = np.linalg.norm(Xb, axis=-1)
        rn = (1.0 / np.maximum(norm, 1e-6)) * validf
        Pmat = np.zeros((U, ncat), np.float32)
        Pmat[np.arange(U), np.clip(spk[b], 0, ncat - 1)] = 1.0
        ptb_a = np.ascontiguousarray(Pmat.T)
        pta_a = np.ascontiguousarray((-gate)[:, None, None] * ptb_a[None])
        m = dict(
            xt=np.ascontiguousarray(Xb.T).reshape(4, P, U),
            xpbo=np.ascontiguousarray((Xb + bo).reshape(8, P, D)),
            rns_a=np.ascontiguousarray((sims[0] * rn)[None, :]),
            rns_b=np.ascontiguousarray(rn[None, :]),
            pta=pta_a,
            ptb=ptb_a,
            validd=np.ascontiguousarray(validf.reshape(8, P).T),
            **shared,
        )
        in_maps.append(m)

    res = run_bass_kernel_spmd(nc, in_maps, core_ids=list(range(NCORES)))
    outs = [r["out"].reshape(U, D) for r in res.results]
    return np.stack(outs).astype(np.float32)


# revision 27
# speedup vs baseline: 6.9474x; 1.0002x over previous
"""Trainium2 Bass kernel for nn_BiasedMHABlock (biased MHA + FFN transformer block).

Sharding: batch B=8 -> one batch per NeuronCore (SPMD, no collectives).

Per-core math (batch b), fully fused on-device:
  scores^T[w,u] per head = (K_h Q_h^T)/8 + CB[w,u] + relband_h[w,u], where
  softmax-invariant constants are dropped and
  CB = simscale*Xn Xn^T - gate*OneHot(spk) OneHot(spk)^T is built once via PE
  and added per-head with identity-matmul PSUM accumulation.
  Softmax runs over the partition axis without max-subtraction (scores are O(1));
  the denominator comes free from an appended ones-column of V in the attn@V
  matmul and is divided out post-hoc.
  Then X1 = LN(X+bo + Attn@Wo), FFN with transposed hidden, X2 = LN(X1+ffn).

All matmuls run as float32r (full-rate fp32); ffn2 runs bf16.
"""
import sys
import math

for _p in ("/opt/trn_rl_repo",):
    if _p not in sys.path:
        sys.path.insert(0, _p)

import numpy as np
import ml_dtypes

import concourse.bass as bass
import concourse.tile as tile
from concourse import bacc, mybir
from concourse.bass_utils import run_bass_kernel_spmd

F32 = mybir.dt.float32
F32R = mybir.dt.float32r
BF16 = mybir.dt.bfloat16
AF = mybir.ActivationFunctionType
ALU = mybir.AluOpType

B, U, D, H, DH, DFF = 8, 1024, 512, 8, 64, 4096
REL_MAX = 128
P = 128
NCORES = 8
LN_EPS = 1e-5
UBLK = 512  # ffn u-block

_prog_cache = {}
SKIP = set()  # perf-analysis only: phase names to skip


def _build_program(fast_gates: bool, apply_mask: bool, ncat: int, ln1_triv: bool = False, ln2_triv: bool = False):
    nc = bacc.Bacc("TRN2", target_bir_lowering=False, debug=False)

    def din(name, shape, dt=F32R):
        return nc.dram_tensor(name, list(shape), dt, kind="ExternalInput").ap()

    xt = din("xt", [4, P, U])
    xpbo = din("xpbo", [8, P, D], F32)
    rns_a = din("rns_a", [1, U], F32)
    rns_b = din("rns_b", [1, U], F32)
    pta = din("pta", [H, ncat, U])
    ptb = din("ptb", [ncat, U])
    wq = din("wq", [4, P, D])
    wk = din("wk", [4, P, D])
    wv = din("wv", [5, P, D])
    wo = din("wo", [4, P, D])
    w1 = din("w1", [4, P, DFF])
    w2 = din("w2", [33, P, D])
    bf1p = din("bf1p", [P, 32], F32)
    qkb = din("qkb", [P, 8], F32)
    rbd = din("rbd", [P, H, 3, P])
    lnw = din("lnw", [4, D], F32)
    expd = din("expd", [4, 2 * P])
    uvec4 = din("uvec4", [1, 16])
    identd = din("identd", [P, P])
    identfd = din("identfd", [P, P], F32)
    ones_pe = din("ones_pe", [1, P])
    ones_v = din("ones_v", [P, 64])
    validd = din("validd", [P, 8], F32)
    if not fast_gates:
        sidents = din("sidents", [H, P, P])
        gidents = din("gidents", [H, P, P])

    out = nc.dram_tensor("out", [8, P, D], F32, kind="ExternalOutput").ap()

    open_cms = {}

    with tile.TileContext(nc) as tc, nc.allow_low_precision(reason="fp32r kernel"):
        def pool(name, bufs, space="SBUF", side="left"):
            cm = tc.tile_pool(name=name, bufs=bufs, space=space, side=side)
            p = cm.__enter__()
            open_cms[name] = cm
            return p

        def close(*names):
            for n in names:
                open_cms.pop(n).__exit__(None, None, None)

        try:
            # ---------------- constants (left, whole-kernel) ----------------
            consts = pool("consts", 1)
            ident = consts.tile([P, P], F32R)
            identf = consts.tile([P, P], F32)
            qkb_t = consts.tile([P, 8], F32)
            bf1_t = consts.tile([P, 32], F32)
            valid_t = consts.tile([P, 8], F32)
            epst = consts.tile([P, 1], F32)
            ones_u = consts.tile([1, P], F32R)
            ones_bf = consts.tile([1, P], F32R)

            # ------------- long-lived attention inputs (left) ----------------
            attn_in = pool("attn_in", 1)
            qt_t = attn_in.tile([P, 4, U], F32R, tag="qt", name="qt")
            kt_t = attn_in.tile([P, 4, U], F32R, tag="kt", name="kt")
            vt_t = attn_in.tile([P, 8, 520], F32R, tag="vt", name="vt")
            rb_t = attn_in.tile([P, H, 3, P], F32R, tag="rbt", name="rbt")
            if fast_gates:
                cb_mats = [attn_in.tile([P, 8, U], F32R, tag="cbt", name="cbt")]
            else:
                cb_mats = [
                    attn_in.tile([P, 8, U], F32R, tag="simt", name="simt"),
                    attn_in.tile([P, 8, U], F32R, tag="eqt", name="eqt"),
                ]
                sid_t = attn_in.tile([P, H, P], F32R, tag="sid", name="sid")
                nc.sync.dma_start(sid_t, sidents.rearrange("h p q -> p h q"))
                gid_t = attn_in.tile([P, H, P], F32R, tag="gid", name="gid")
                nc.sync.dma_start(gid_t, gidents.rearrange("h p q -> p h q"))

            # ======================= PHASE 1: prep ==========================
            pre = pool("pre", 1, side="right")
            prew = pool("prew", 2, side="right")
            ps1 = pool("ps1", 2, space="PSUM")

            xt_t = pre.tile([P, 4, U], F32R, tag="xt", name="xtt")
            nc.sync.dma_start(xt_t[:, 0, 0:D], xt[0][:, 0:D])
            wq_t = prew.tile([P, 5, D], F32R, tag="wx", name="wqt")
            wk_t = prew.tile([P, 5, D], F32R, tag="wx", name="wkt")
            nc.sync.dma_start(wq_t[:, 0, :], wq[0])
            for c in range(1, 4):
                nc.sync.dma_start(xt_t[:, c, 0:D], xt[c][:, 0:D])
                nc.sync.dma_start(wq_t[:, c, :], wq[c])
            for c in range(4):
                nc.sync.dma_start(xt_t[:, c, D:U], xt[c][:, D:U])
                nc.sync.dma_start(wk_t[:, c, :], wk[c])
            nc.sync.dma_start(qkb_t, qkb)
            if fast_gates:
                rnsa_b = pre.tile([P, U], F32, tag="rnsa", name="rnsa")
                nc.gpsimd.dma_start(
                    rnsa_b,
                    bass.AP(tensor=rns_a.tensor, offset=0, ap=[[0, P], [1, U]]),
                )
            rnsb_b = pre.tile([P, U], F32, tag="rnsb", name="rnsb")
            nc.gpsimd.dma_start(
                rnsb_b, bass.AP(tensor=rns_b.tensor, offset=0, ap=[[0, P], [1, U]])
            )
            nc.sync.dma_start(ident, identd)
            nc.sync.dma_start(identf, identfd)
            nc.sync.dma_start(bf1_t, bf1p)
            nc.sync.dma_start(valid_t, validd)
            nc.vector.memset(epst, LN_EPS)
            nc.sync.dma_start(ones_u, ones_pe)
            nc.sync.dma_start(ones_bf, ones_pe)

            # Q^T, K^T: psum[e_tile, u_half] = sum_c Wx[c]-slice^T @ XT
            for (wt, dst, boff) in () if "qkproj" in SKIP else ((wq_t, qt_t, 0), (wk_t, kt_t, 4)):
                for t in range(4):
                    for j in range(2):
                        ps = ps1.tile([P, D], F32, tag="psqk", name="psqk")
                        for c in range(4):
                            nc.tensor.matmul(
                                ps,
                                wt[:, c, t * P:(t + 1) * P],
                                xt_t[:, c, j * D:(j + 1) * D],
                                start=(c == 0), stop=(c == 3),
                            )
                        nc.scalar.activation(
                            dst[:, t, j * D:(j + 1) * D], ps, AF.Identity,
                            bias=qkb_t[:, boff + t:boff + t + 1],
                        )

            # V (interleaved (dh h) layout + ones cols)
            wv_t = prew.tile([P, 5, D], F32R, tag="wx", name="wvt")
            for c in range(5):
                nc.sync.dma_start(wv_t[:, c, :], wv[c])
            for t in range(0 if "vproj" in SKIP else 8):
                ps = ps1.tile([P, D], F32, tag="psv", name="psv")
                for c in range(4):
                    nc.tensor.matmul(
                        ps, xt_t[:, c, t * P:(t + 1) * P], wv_t[:, c, :],
                        start=(c == 0), stop=False,
                    )
                nc.tensor.matmul(
                    ps, ones_u[0:1, :], wv_t[0:1, 4, :],
                    start=False, stop=True,
                )
                nc.vector.tensor_copy(
                    vt_t[:, t, 0:D].rearrange("p (dh h) -> p dh h", h=H),
                    ps.rearrange("p (h dh) -> p dh h", h=H),
                )
            nc.sync.dma_start(
                vt_t[:, :, D:520], ones_v.rearrange("p (t f) -> p t f", t=8)
            )
            if apply_mask:
                for t in range(8):
                    nc.vector.tensor_scalar_mul(
                        vt_t[:, t, :], vt_t[:, t, :], valid_t[:, t:t + 1],
                    )

            # Xn^T (scaled / unscaled sides) and CB (or SIM + EQ)
            if fast_gates:
                xna_t = pre.tile([P, 4, U], F32R, tag="xna", name="xna")
            xnb_t = pre.tile([P, 4, U], F32R, tag="xnb", name="xnb")
            for c in range(4):
                if fast_gates:
                    nc.vector.tensor_tensor(
                        xna_t[:, c, :], xt_t[:, c, :], rnsa_b, ALU.mult
                    )
                nc.vector.tensor_tensor(
                    xnb_t[:, c, :], xt_t[:, c, :], rnsb_b, ALU.mult
                )

            ptb_t = pre.tile([ncat, U], F32R, tag="ptb", name="ptbt")
            nc.sync.dma_start(ptb_t, ptb)
            if fast_gates:
                pta_t = pre.tile([ncat, 1, U], F32R, tag="pta", name="ptat")
                nc.sync.dma_start(pta_t[:, 0, :], pta[0])

            if fast_gates:
                cbt = cb_mats[0]
                for i in range(0 if "cb" in SKIP else 8):
                    for j in range(2):
                        ps = ps1.tile([P, D], F32, tag="pscb", name="pscb")
                        for c in range(4):
                            nc.tensor.matmul(
                                ps,
                                xna_t[:, c, i * P:(i + 1) * P],
                                xnb_t[:, c, j * D:(j + 1) * D],
                                start=(c == 0), stop=False,
                            )
                        nc.tensor.matmul(
                            ps,
                            pta_t[:, 0, i * P:(i + 1) * P],
                            ptb_t[:, j * D:(j + 1) * D],
                            start=False, stop=True,
                        )
                        nc.vector.tensor_copy(
                            cbt[:, i, j * D:(j + 1) * D], ps
                        )
            else:
                simt, eqt = cb_mats
                for i in range(8):
                    for j in range(2):
                        ps = ps1.tile([P, D], F32, tag="pscb", name="pscb")
                        for c in range(4):
                            nc.tensor.matmul(
                                ps,
                                xnb_t[:, c, i * P:(i + 1) * P],
                                xnb_t[:, c, j * D:(j + 1) * D],
                                start=(c == 0), stop=(c == 3),
                            )
                        nc.scalar.activation(
                            simt[:, i, j * D:(j + 1) * D], ps, AF.Copy
                        )
                        ps2 = ps1.tile([P, D], F32, tag="pscb", name="pscb2")
                        nc.tensor.matmul(
                            ps2,
                            ptb_t[:, i * P:(i + 1) * P],
                            ptb_t[:, j * D:(j + 1) * D],
                            start=True, stop=True,
                        )
                        nc.scalar.activation(
                            eqt[:, i, j * D:(j + 1) * D], ps2, AF.Copy
                        )

            close("ps1", "prew", "pre")

            nc.sync.dma_start(rb_t, rbd)

            # ====================== PHASE 2: attention ======================
            mid = pool("mid", 1, side="right")
            attnT = mid.tile([P, 4, U], F32R, tag="attnT", name="attnT")
            den_sb = mid.tile([1, 16, D], F32R, tag="densb", name="densb")
            expd_t = mid.tile([4, 2 * P], F32R, tag="expd", name="expdt")
            nc.sync.dma_start(expd_t, expd)
            uvec_t = mid.tile([1, 16], F32R, tag="uvec", name="uvect")
            nc.sync.dma_start(uvec_t, uvec4)

            epool = pool("epool", 4, side="right")
            dnp = pool("dnp", 4, side="right")
            pss = pool("pss", 4, space="PSUM")
            psa = pool("psa", 4, space="PSUM")

            for h in range(0 if "attn" in SKIP else H):
                po = (h % 2) * 64
                ch = h // 2
                patts = [
                    psa.tile([65, D], F32, tag="psatt", name=f"psatt_{h}_{j}")
                    for j in range(2)
                ]
                for i in range(8):
                    et = epool.tile([P, U], F32R, tag="et", name="et")
                    for j in range(2):
                        ps = pss.tile([P, D], F32, tag="pssc", name="pssc")
                        mms = [(
                            kt_t[po:po + 64, ch, i * P:(i + 1) * P],
                            qt_t[po:po + 64, ch, j * D:(j + 1) * D],
                            slice(0, D),
                        )]
                        if fast_gates:
                            adds = [(ident, cb_mats[0])]
                        else:
                            adds = [(sid_t[:, h, :], cb_mats[0]),
                                    (gid_t[:, h, :], cb_mats[1])]
                        for (idm, mat) in adds:
                            mms.append((
                                idm,
                                mat[:, i, j * D:(j + 1) * D],
                                slice(0, D),
                            ))
                        # banded rel bias: blocks i-1, i, i+1, clipped to half j
                        lo_b = max(i - 1, 0)
                        hi_b = min(i + 1, 7)
                        run_lo = max(lo_b * P, j * D)
                        run_hi = min((hi_b + 1) * P, (j + 1) * D)
                        if run_hi > run_lo:
                            o0 = (run_lo // P) - (i - 1)
                            o1 = (run_hi // P) - (i - 1)
                            mms.append((
                                ident,
                                rb_t[:, h, o0:o1, :],
                                slice(run_lo - j * D, run_hi - j * D),
                            ))
                        for mi, (lhsT, rhs, osl) in enumerate(mms):
                            nc.tensor.matmul(
                                ps[:, osl], lhsT, rhs,
                                start=(mi == 0), stop=(mi == len(mms) - 1),
                                skip_group_check=True,
                            )
                        nc.scalar.activation(
                            et[:, j * D:(j + 1) * D], ps, AF.Exp
                        )
                    for j in range(2):
                        nc.tensor.matmul(
                            patts[j],
                            vt_t[:, i, h:h + 513:8],
                            et[:, j * D:(j + 1) * D],
                            start=(i == 0), stop=(i == 7),
                        )
                for j in range(2):
                    idx = h * 2 + j
                    nc.vector.tensor_copy(
                        den_sb[0:1, idx, :], patts[j][64:65, :]
                    )
                    nc.vector.tensor_copy(
                        attnT[po:po + 64, ch, j * D:(j + 1) * D],
                        patts[j][0:64, :],
                    )
                if h % 2 == 1:
                    # normalize chunk ch: heads 2ch, 2ch+1 are done
                    c4 = 4 * ch
                    psg = psa.tile([4, D], F32, tag="psatt", name=f"psg_{ch}")
                    for r in range(4):
                        nc.tensor.matmul(
                            psg,
                            uvec_t[0:1, r * 4:(r + 1) * 4],
                            den_sb[0:1, c4 + r, :],
                            start=(r == 0), stop=(r == 3),
                        )
                    rden4 = dnp.tile([4, D], F32R, tag="rden4", name="rden4")
                    nc.vector.reciprocal(rden4, psg)
                    for j in range(2):
                        psn = psa.tile([P, D], F32, tag="psatt", name=f"psn_{ch}_{j}")
                        nc.tensor.matmul(
                            psn,
                            expd_t[:, j * P:(j + 1) * P],
                            rden4,
                            start=True, stop=True,
                        )
                        nc.vector.tensor_tensor(
                            attnT[:, ch, j * D:(j + 1) * D],
                            attnT[:, ch, j * D:(j + 1) * D],
                            psn, ALU.mult,
                        )

            close("psa", "pss", "dnp", "epool")
            close("attn_in")

            # ---------- x1 pool opens early on the left (outlives mid) -------
            x1p = pool("x1p", 1)
            x1_t = x1p.tile([P, 8, D], F32, tag="x1", name="x1")
            x1T_t = x1p.tile([P, 4, U], F32R, tag="x1T", name="x1T")
            lnwb = None
            if not (ln1_triv and ln2_triv):
                lnwb = x1p.tile([P, 4, D], F32, tag="lnwb", name="lnwb")
                for k in range(4):
                    src = bass.AP(tensor=lnw.tensor, offset=k * D,
                                  ap=[[0, P], [1, D]])
                    nc.gpsimd.dma_start(lnwb[:, k, :], src)


            # ======================= PHASE 3: X1 = LN1 ======================
            x1w = pool("x1w", 1, side="right")
            lns = pool("lns", 8, side="right")
            psc = pool("psc", 3, space="PSUM")
            pst = pool("pst", 3, space="PSUM")

            wo_t = x1w.tile([P, 4, D], F32R, tag="wo", name="wot")
            for c in range(4):
                nc.sync.dma_start(wo_t[:, c, :], wo[c])
            xpbo_t = x1w.tile([P, 8, D], F32, tag="xpbo", name="xpbot")
            for t in range(8):
                nc.sync.dma_start(xpbo_t[:, t, :], xpbo[t])

            for t in range(0 if "x1" in SKIP else 8):
                ps = psc.tile([P, D], F32, tag="psx1", name="psx1")
                for c in range(4):
                    nc.tensor.matmul(
                        ps,
                        attnT[:, c, t * P:(t + 1) * P],
                        wo_t[:, c, :],
                        start=(c == 0), stop=(c == 3),
                    )
                o1 = lns.tile([P, D], F32, tag="o1", name="o1")
                nc.vector.tensor_tensor(o1, ps, xpbo_t[:, t, :], ALU.add)
                ps = o1
                stats = lns.tile([P, 6], F32, tag="st", name="st")
                nc.vector.bn_stats(stats, ps)
                mv = lns.tile([P, 2], F32, tag="mv", name="mv")
                nc.vector.bn_aggr(mv, stats)
                rstd = lns.tile([P, 1], F32, tag="rstd", name="rstd")
                nc.scalar.activation(rstd, mv[:, 1:2], AF.Sqrt, bias=epst)
                nc.vector.reciprocal(rstd, rstd)
                if ln1_triv and not apply_mask:
                    nc.vector.tensor_scalar(
                        x1_t[:, t, :], ps, mv[:, 0:1], rstd,
                        ALU.subtract, ALU.mult,
                    )
                elif ln1_triv:
                    xh = lns.tile([P, D], F32, tag="xh", name="xh")
                    nc.vector.tensor_scalar(
                        xh, ps, mv[:, 0:1], rstd, ALU.subtract, ALU.mult
                    )
                    nc.vector.tensor_scalar_mul(
                        x1_t[:, t, :], xh, valid_t[:, t:t + 1],
                    )
                else:
                    xh = lns.tile([P, D], F32, tag="xh", name="xh")
                    nc.vector.tensor_scalar(
                        xh, ps, mv[:, 0:1], rstd, ALU.subtract, ALU.mult
                    )
                    xg = lns.tile([P, D], F32, tag="xg", name="xg")
                    nc.vector.tensor_tensor(xg, xh, lnwb[:, 0, :], ALU.mult)
                    if apply_mask:
                        nc.vector.tensor_tensor(xg, xg, lnwb[:, 1, :], ALU.add)
                        nc.vector.tensor_scalar_mul(
                            x1_t[:, t, :], xg, valid_t[:, t:t + 1],
                        )
                    else:
                        nc.vector.tensor_tensor(
                            x1_t[:, t, :], xg, lnwb[:, 1, :], ALU.add
                        )
                for c in range(4):
                    pt = pst.tile([P, P], F32, tag="pstr", name="pstr")
                    nc.tensor.transpose(
                        pt, x1_t[:, t, c * P:(c + 1) * P], identf
                    )
                    nc.scalar.activation(
                        x1T_t[:, c, t * P:(t + 1) * P], pt, AF.Copy
                    )

            close("pst", "psc", "lns", "x1w")
            close("mid")

            # ========================= PHASE 4: FFN =========================
            ffnw = pool("ffnw", 1)  # left stack: consts, x1p, ffnw
            hidp = pool("hidp", 1, side="right")
            w2s = pool("w2s", 6, side="right")
            lns2 = pool("lns2", 8, side="right")
            outp = pool("outp", 2, side="right")
            psf = pool("psf", 4, space="PSUM")

            w1_t = ffnw.tile([P, 4, DFF], F32R, tag="w1", name="w1t")
            for c in range(4):
                nc.sync.dma_start(w1_t[:, c, :], w1[c])

            ublk = UBLK if (ln1_triv and ln2_triv and not apply_mask) else 256
            nblk = 0 if "ffn" in SKIP else U // ublk
            for ub in range(nblk):
                hid = hidp.tile([P, 32, ublk], F32R, tag="hid", name="hid")
                for t in range(32):
                    ps = psf.tile([P, ublk], F32, tag="psh", name="psh")
                    for c in range(4):
                        nc.tensor.matmul(
                            ps,
                            w1_t[:, c, t * P:(t + 1) * P],
                            x1T_t[:, c, ub * ublk:(ub + 1) * ublk],
                            start=(c == 0), stop=(c == 3),
                        )
                    nc.scalar.activation(
                        hid[:, t, :], ps, AF.Relu, bias=bf1_t[:, t:t + 1],
                    )
                nv = ublk // P
                psos = [
                    psf.tile([P, D], F32, tag="pso", name=f"pso{v}")
                    for v in range(nv)
                ]
                for c in range(33):
                    w2c = w2s.tile([P, D], F32R, tag="w2c", name="w2c")
                    nc.sync.dma_start(w2c, w2[c])
                    for v in range(nv):
                        if c < 32:
                            nc.tensor.matmul(
                                psos[v],
                                hid[:, c, v * P:(v + 1) * P],
                                w2c,
                                start=(c == 0), stop=False,
                                skip_group_check=True,
                            )
                        else:
                            nc.tensor.matmul(
                                psos[v], ones_bf, w2c[0:1, :],
                                start=False, stop=True, skip_group_check=True,
                            )
                for v in range(nv):
                    g = ub * nv + v
                    ps = psos[v]
                    x2p = lns2.tile([P, D], F32, tag="x2p", name="x2p")
                    nc.vector.tensor_tensor(x2p, ps, x1_t[:, g, :], ALU.add)
                    ps = x2p
                    stats = lns2.tile([P, 6], F32, tag="st2", name="st2")
                    nc.vector.bn_stats(stats, ps)
                    mv = lns2.tile([P, 2], F32, tag="mv2", name="mv2")
                    nc.vector.bn_aggr(mv, stats)
                    rstd = lns2.tile([P, 1], F32, tag="rstd2", name="rstd2")
                    nc.scalar.activation(rstd, mv[:, 1:2], AF.Sqrt, bias=epst)
                    nc.vector.reciprocal(rstd, rstd)
                    x2 = outp.tile([P, D], F32, tag="x2", name="x2")
                    if ln2_triv and not apply_mask:
                        nc.vector.tensor_scalar(
                            x2, ps, mv[:, 0:1], rstd, ALU.subtract, ALU.mult
                        )
                        nc.sync.dma_start(out[g], x2)
                    elif ln2_triv:
                        xh = lns2.tile([P, D], F32, tag="xh2", name="xh2")
                        nc.vector.tensor_scalar(
                            xh, ps, mv[:, 0:1], rstd, ALU.subtract, ALU.mult
                        )
                        nc.vector.tensor_scalar_mul(x2, xh, valid_t[:, g:g + 1])
                        nc.sync.dma_start(out[g], x2)
                    else:
                        xh = lns2.tile([P, D], F32, tag="xh2", name="xh2")
                        nc.vector.tensor_scalar(
                            xh, ps, mv[:, 0:1], rstd, ALU.subtract, ALU.mult
                        )
                        xg = lns2.tile([P, D], F32, tag="xg2", name="xg2")
                        nc.vector.tensor_tensor(xg, xh, lnwb[:, 2, :], ALU.mult)
                        if apply_mask:
                            nc.vector.tensor_tensor(xg, xg, lnwb[:, 3, :], ALU.add)
                            nc.vector.tensor_scalar_mul(
                                x2, xg, valid_t[:, g:g + 1]
                            )
                        else:
                            nc.vector.tensor_tensor(x2, xg, lnwb[:, 3, :], ALU.add)
                        nc.sync.dma_start(out[g], x2)

            close("psf", "outp", "lns2", "w2s", "hidp", "ffnw", "x1p", "consts")
        finally:
            for n in list(open_cms):
                try:
                    open_cms.pop(n).__exit__(None, None, None)
                except Exception:
                    pass

    nc.compile()
    return nc


def _get_program(fast_gates, apply_mask, ncat, ln1_triv=False, ln2_triv=False):
    key = (fast_gates, apply_mask, ncat, ln1_triv, ln2_triv)
    if key not in _prog_cache:
        _prog_cache[key] = _build_program(fast_gates, apply_mask, ncat,
                                          ln1_triv, ln2_triv)
    return _prog_cache[key]


def kernel(**inputs):
    X = np.ascontiguousarray(np.asarray(inputs["X"], dtype=np.float32))
    mask = np.asarray(inputs["mask_u"]).astype(bool)
    spk = np.asarray(inputs["speakers"]).astype(np.int64)
    Wq = np.asarray(inputs["Wq"], np.float32); bq = np.asarray(inputs["bq"], np.float32)
    Wk = np.asarray(inputs["Wk"], np.float32); bk = np.asarray(inputs["bk"], np.float32)
    Wv = np.asarray(inputs["Wv"], np.float32); bv = np.asarray(inputs["bv"], np.float32)
    Wo = np.asarray(inputs["Wo"], np.float32); bo = np.asarray(inputs["bo"], np.float32)
    relb = np.asarray(inputs["rel_bias"], np.float32)
    gate = np.asarray(inputs["speaker_gate"], np.float32)
    sims = np.asarray(inputs["sim_scale"], np.float32)
    g1 = np.asarray(inputs["g1"], np.float32); beta1 = np.asarray(inputs["beta1"], np.float32)
    g2 = np.asarray(inputs["g2"], np.float32); beta2 = np.asarray(inputs["beta2"], np.float32)
    W1 = np.asarray(inputs["W1"], np.float32); bf1 = np.asarray(inputs["bf1"], np.float32)
    W2 = np.asarray(inputs["W2"], np.float32); bf2 = np.asarray(inputs["bf2"], np.float32)

    ncat = int(max(9, spk.max() + 1))
    fast_gates = bool(np.all(gate == gate[0]) and np.all(sims == sims[0]))
    apply_mask = not bool(mask.all())

    ln1_triv = bool(np.all(g1 == 1.0) and np.all(beta1 == 0.0))
    ln2_triv = bool(np.all(g2 == 1.0) and np.all(beta2 == 0.0))
    nc = _get_program(fast_gates, apply_mask, ncat, ln1_triv, ln2_triv)

    # ---- shared (weight) arrays ----
    scale = 1.0 / math.sqrt(DH)
    wq_a = np.ascontiguousarray((Wq * scale).reshape(4, P, D))
    wk_a = np.ascontiguousarray(Wk.reshape(4, P, D))
    wv_a = np.concatenate([Wv.reshape(4, P, D), np.zeros((1, P, D), np.float32)], 0)
    wv_a[4, 0, :] = bv
    wv_a = np.ascontiguousarray(wv_a)
    wo_a = np.ascontiguousarray(Wo.reshape(4, P, D))
    w1_a = np.ascontiguousarray(W1.reshape(4, P, DFF))
    w2_a = np.concatenate([W2.reshape(32, P, D), np.zeros((1, P, D), np.float32)], 0)
    w2_a[32, 0, :] = bf2
    w2_a = np.ascontiguousarray(w2_a)
    bf1p_a = np.ascontiguousarray(bf1.reshape(32, P).T)
    qkb_a = np.zeros((P, 8), np.float32)
    qkb_a[:, 0:4] = (bq * scale).reshape(4, P).T
    qkb_a[:, 4:8] = bk.reshape(4, P).T
    lnw_a = np.ascontiguousarray(np.stack([g1, beta1, g2, beta2]))

    # banded rel bias: rb[a, h, o, c] = relb[h, min(|(o-1)*128+c-a|,128)] - relb[h,128]
    a_i = np.arange(P)[:, None]
    c_i = np.arange(P)[None, :]
    rb_hoc = np.zeros((H, 3, P, P), np.float32)
    for o in range(3):
        dist = np.minimum(np.abs((o - 1) * P + c_i - a_i), REL_MAX)
        rb_hoc[:, o] = relb[:, dist] - relb[:, REL_MAX][:, None, None]
    rbd_a = np.ascontiguousarray(rb_hoc.transpose(2, 0, 1, 3))  # [a, h, o, c]

    # denominator-broadcast expander: r = (h - 2c)*2 + j
    expd_a = np.zeros((4, 2, P), np.float32)
    for j in range(2):
        expd_a[j, j, 0:64] = 1.0
        expd_a[2 + j, j, 64:P] = 1.0
    expd_a = np.ascontiguousarray(expd_a.reshape(4, 2 * P))

    ident_a = np.eye(P, dtype=np.float32)
    uvec4_a = np.ascontiguousarray(np.eye(4, dtype=np.float32).reshape(1, 16))

    shared = dict(wq=wq_a, wk=wk_a, wv=wv_a, wo=wo_a, w1=w1_a, w2=w2_a,
                  bf1p=bf1p_a, qkb=qkb_a, lnw=lnw_a, rbd=rbd_a, expd=expd_a,
                  identd=ident_a, identfd=ident_a, uvec4=uvec4_a,
                  ones_pe=np.ones((1, P), np.float32),
                  ones_v=np.ones((P, 64), np.float32))
    if not fast_gates:
        shared["sidents"] = np.ascontiguousarray(sims[:, None, None] * ident_a[None])
        shared["gidents"] = np.ascontiguousarray(-gate[:, None, None] * ident_a[None])

    in_maps = []
    for b in range(B):
        Xb = X[b]
        validf = mask[b].astype(np.float32)
        norm = np.linalg.norm(Xb, axis=-1)
        rn = (1.0 / np.maximum(norm, 1e-6)) * validf
        Pmat = np.zeros((U, ncat), np.float32)
        Pmat[np.arange(U), np.clip(spk[b], 0, ncat - 1)] = 1.0
        ptb_a = np.ascontiguousarray(Pmat.T)
        pta_a = np.ascontiguousarray((-gate)[:, None, None] * ptb_a[None])
        m = dict(
            xt=np.ascontiguousarray(Xb.T).reshape(4, P, U),
            xpbo=np.ascontiguousarray((Xb + bo).reshape(8, P, D)),
            rns_a=np.ascontiguousarray((sims[0] * rn)[None, :]),
            rns_b=np.ascontiguousarray(rn[None, :]),
            pta=pta_a,
            ptb=ptb_a,
            validd=np.ascontiguousarray(validf.reshape(8, P).T),
            **shared,
        )
        in_maps.append(m)

    res = run_bass_kernel_spmd(nc, in_maps, core_ids=list(range(NCORES)))
    outs = [r["out"].reshape(U, D) for r in res.results]
    return np.stack(outs).astype(np.float32)
